# revision 7
# baseline (speedup 1.0000x reference)
"""nn_Conv2d_BN_int8 TRN2 Bass kernel.

Reference computation (see harness reference.py):
    q_x = clip(round(x / scale_x), -127, 127)                    per-tensor int8
    scale_w = max|w| over (C,kH,kW) / 127;  q_w = clip(round(w / scale_w))
    acc = conv2d(q_x, q_w, 3x3, stride 1, pad 1)   (fp32 accumulation, integer-valued)
    out = acc * scale_x * scale_w[o] + bias[o]

Strategy:
  - Data-parallel over batch: 32 images -> 4 per core x 8 cores. Each core
    holds the full weights (1.2 MB).
  - int8 values in [-127,127] are exactly representable in bf16, and the PE
    accumulates in fp32 -> a bf16 GEMM of the quantized tensors is an exact
    integer GEMM (partial sums ~1e5 << 2^24).
  - conv 3x3/pad1 == sum over 9 taps of shifted GEMMs: K=C=128 (partition
    dim), M=O tile of 128, N=448 (8 output rows x 56 cols) accumulated in
    one PSUM bank over the 9 taps.
  - x is quantized on-device (DVE; round-to-nearest-even via the +1.5*2^23
    magic-number trick, matching jnp.round) into a zero-padded [128,58,58]
    bf16 image so every tap is a strided AP, no boundary cases.
  - w is quantized on-device in O-major layout (per-channel absmax reduce),
    then moved to C-major [c, tap, o] via 9 PE transposes per o-tile; the
    o-tile-1 prep is deferred until after the img0/o-tile-0 matmuls are
    emitted so the PE starts real work as early as possible.
  - dequant + bias fused into one DVE tensor_scalar pass (per-partition
    scale/bias APs) during PSUM eviction; output DMA per 8-row block.
"""

import numpy as np

import concourse.bacc as bacc
import concourse.mybir as mybir
import concourse.tile as tile
from concourse.bass_utils import run_bass_kernel_spmd
from concourse.masks import make_identity

B, C, H, W = 32, 128, 56, 56
O, KH, KW = 256, 3, 3
NCORES = 8
BPC = B // NCORES            # images per core
HP, WP = H + 2, W + 2        # padded image
RB = 8                       # output rows per matmul block
NBLK = H // RB               # 7 blocks
NFREE = RB * W               # 448 <= 512 (one PSUM bank)
NTAP = KH * KW               # 9
MAGIC = 12582912.0           # 1.5 * 2**23: fp32 RNE integer rounding trick
QMAX = 127.0

f32 = mybir.dt.float32
bf16 = mybir.dt.bfloat16


def build(inv_sx: float, sx: float):
    """Emit the per-core Bass program. inv_sx = 1/scale_x, sx = scale_x."""
    nc = bacc.Bacc(
        "TRN2",
        target_bir_lowering=False,
        debug=False,
        enable_asserts=False,
        num_devices=NCORES,
    )
    x_d = nc.dram_tensor("x", [BPC, C, H, W], f32, kind="ExternalInput").ap()
    w_d = nc.dram_tensor("w", [O, C, KH, KW], f32, kind="ExternalInput").ap()
    b_d = nc.dram_tensor("b", [O], f32, kind="ExternalInput").ap()
    out_d = nc.dram_tensor("out", [BPC, O, H, W], f32, kind="ExternalOutput").ap()

    # second HWDGE ring (qActDynamicHW) for input loads; outputs go on sync
    in_dma = nc.scalar if hasattr(nc.scalar, "dma_start") else nc.sync

    with tile.TileContext(nc) as tc:
        with (
            tc.tile_pool(name="singles", bufs=1) as singles,
            tc.tile_pool(name="wtmp", bufs=2) as wtmp,
            tc.tile_pool(name="wsc", bufs=8) as wsc,
            tc.tile_pool(name="qwp", bufs=2) as qwp,
            tc.tile_pool(name="xsp", bufs=3) as xsp,
            tc.tile_pool(name="xpadp", bufs=3) as xpadp,
            tc.tile_pool(name="outp", bufs=3) as outp,
            tc.tile_pool(name="psum", bufs=8, space="PSUM") as psump,
        ):
            idn = singles.tile([128, 128], bf16, name="idn")
            make_identity(nc, idn)

            # HAM warmup: ~40 dummy matmuls (idn @ idn) keep the PE busy
            # from ~t=0 so the clock gate is at 8/8 when real matmuls start
            wps = psump.tile([128, 128], f32, name="wps", tag="ps")
            for _ in range(40):
                nc.tensor.matmul(wps, idn, idn, start=True, stop=True)

            bias_sb = singles.tile([128, 2], f32, name="bias_sb")
            nc.sync.dma_start(out=bias_sb, in_=b_d.rearrange("(t p) -> p t", p=128))

            # [c, tap, o] bf16 stationary weights for the GEMMs
            wmm = singles.tile([128, NTAP, O], bf16, name="wmm")
            # per-o dequant scale (scale_w * scale_x), partition layout
            dqs = singles.tile([128, 2], f32, name="dqs")

            # weight loads first on the sync ring (startup critical path);
            # x loads go on the scalar HWDGE ring so they never queue
            # behind weights or output stores
            w_flat = w_d.rearrange("o c kh kw -> o (c kh kw)")  # [256, 1152]
            wf = []
            for t in range(2):
                wf_t = wtmp.tile([128, C * NTAP], f32, name=f"wf{t}")
                nc.sync.dma_start(out=wf_t, in_=w_flat[t * 128 : (t + 1) * 128])
                wf.append(wf_t)

            x_flat = x_d.rearrange("b c h w -> b c (h w)")      # [BPC, 128, 3136]
            out_flat = out_d.rearrange("b o h w -> b o (h w)")  # [BPC, 256, 3136]
            xs0 = xsp.tile([128, H * W], f32, name="xs")
            in_dma.dma_start(out=xs0, in_=x_flat[0])

            def prep_weights(t):
                """absmax -> 1/scale_w (Newton-refined) -> quantize -> PE
                transpose into wmm[:, :, t*128:(t+1)*128]."""
                mx = wsc.tile([128, 1], f32, name="mx")
                nc.vector.tensor_reduce(
                    out=mx, in_=wf[t], axis=mybir.AxisListType.X,
                    op=mybir.AluOpType.max, apply_absolute_value=True,
                )
                # rq = 127/maxabs, one Newton step: r1 = r0*(2 - mx*r0)
                r0 = wsc.tile([128, 1], f32, name="r0")
                nc.vector.reciprocal(out=r0, in_=mx)
                nt = wsc.tile([128, 1], f32, name="nt")
                nc.vector.tensor_mul(out=nt, in0=mx, in1=r0)
                nc.vector.tensor_scalar(
                    out=nt, in0=nt, scalar1=-1.0, scalar2=2.0,
                    op0=mybir.AluOpType.mult, op1=mybir.AluOpType.add,
                )
                rq = wsc.tile([128, 1], f32, name="rq")
                nc.vector.tensor_mul(out=rq, in0=r0, in1=nt)
                nc.vector.tensor_scalar(
                    out=rq, in0=rq, scalar1=QMAX, scalar2=None,
                    op0=mybir.AluOpType.mult,
                )
                # dequant scale = scale_w * scale_x = maxabs * (scale_x/127)
                nc.vector.tensor_scalar(
                    out=dqs[:, t : t + 1], in0=mx,
                    scalar1=float(np.float32(sx) / np.float32(QMAX)), scalar2=None,
                    op0=mybir.AluOpType.mult,
                )
                # q_w = clip(rne(w / s_w), -127, 127); rne via magic add/sub.
                # mult and magic-add must be separate instructions: a fused
                # tensor_scalar skips the intermediate fp32 rounding, which
                # changes which values sit on .5 rounding boundaries.
                nc.vector.tensor_scalar(
                    out=wf[t], in0=wf[t], scalar1=rq, scalar2=None,
                    op0=mybir.AluOpType.mult,
                )
                nc.vector.tensor_scalar(
                    out=wf[t], in0=wf[t], scalar1=MAGIC, scalar2=None,
                    op0=mybir.AluOpType.add,
                )
                nc.vector.tensor_scalar(
                    out=wf[t], in0=wf[t], scalar1=MAGIC, scalar2=QMAX,
                    op0=mybir.AluOpType.subtract, op1=mybir.AluOpType.min,
                )
                qw = qwp.tile([128, C * NTAP], bf16, name="qw")
                nc.vector.tensor_scalar(
                    out=qw, in0=wf[t], scalar1=-QMAX, scalar2=None,
                    op0=mybir.AluOpType.max,
                )
                # transpose [o, c] -> [c, o] per tap via the PE
                qw3 = qw.rearrange("o (c t) -> o c t", t=NTAP)
                for tap in range(NTAP):
                    tp = psump.tile([128, 128], bf16, name="tp", tag="ps")
                    nc.tensor.transpose(tp, qw3[:, :, tap], idn)
                    nc.scalar.copy(
                        out=wmm[:, tap, t * 128 : (t + 1) * 128], in_=tp
                    )

            prep_weights(0)

            for img in range(BPC):
                if img == 0:
                    xs = xs0
                else:
                    xs = xsp.tile([128, H * W], f32, name="xs")
                    in_dma.dma_start(out=xs, in_=x_flat[img])
                xp = xpadp.tile([128, HP, WP], bf16, name="xp")
                nc.gpsimd.memset(xp[:, 0, :], 0.0)
                nc.gpsimd.memset(xp[:, HP - 1, :], 0.0)
                nc.gpsimd.memset(xp[:, :, 0], 0.0)
                nc.gpsimd.memset(xp[:, :, WP - 1], 0.0)
                # q = clip(rne(x * (1/s_x)), -127, 127), in half-image
                # chunks so the first conv matmuls can start sooner.
                # mult / magic-add as separate instructions (see above).
                xs3 = xs.rearrange("c (h w) -> c h w", w=W)
                HH = H // 2
                for hh in range(2):
                    xsh = xs3[:, hh * HH : (hh + 1) * HH, :]
                    nc.vector.tensor_scalar(
                        out=xsh, in0=xsh, scalar1=inv_sx, scalar2=None,
                        op0=mybir.AluOpType.mult,
                    )
                    nc.vector.tensor_scalar(
                        out=xsh, in0=xsh, scalar1=MAGIC, scalar2=None,
                        op0=mybir.AluOpType.add,
                    )
                    nc.vector.tensor_scalar(
                        out=xsh, in0=xsh, scalar1=MAGIC, scalar2=QMAX,
                        op0=mybir.AluOpType.subtract, op1=mybir.AluOpType.min,
                    )
                    nc.vector.tensor_scalar(
                        out=xp[:, 1 + hh * HH : 1 + (hh + 1) * HH, 1 : W + 1],
                        in0=xsh,
                        scalar1=-QMAX, scalar2=None,
                        op0=mybir.AluOpType.max,
                    )
                for t in range(2):
                    if img == 0 and t == 1:
                        # deferred so img0/t0 matmuls precede it in the PE
                        # queue; ready well before its own matmuls need it
                        prep_weights(1)
                    ot = outp.tile([128, H * W], f32, name="ot")
                    for blk in range(NBLK):
                        ps = psump.tile([128, NFREE], f32, name="ps", tag="ps")
                        for tap in range(NTAP):
                            dy, dx = divmod(tap, KW)
                            nc.tensor.matmul(
                                ps,
                                wmm[:, tap, t * 128 : (t + 1) * 128],
                                xp[:, blk * RB + dy : blk * RB + dy + RB, dx : dx + W],
                                start=(tap == 0),
                                stop=(tap == NTAP - 1),
                            )
                        # out = acc * (s_x*s_w[o]) + bias[o]  (DVE evict)
                        nc.vector.tensor_scalar(
                            out=ot[:, blk * NFREE : (blk + 1) * NFREE],
                            in0=ps,
                            scalar1=dqs[:, t : t + 1],
                            scalar2=bias_sb[:, t : t + 1],
                            op0=mybir.AluOpType.mult,
                            op1=mybir.AluOpType.add,
                        )
                        nc.sync.dma_start(
                            out=out_flat[
                                img, t * 128 : (t + 1) * 128,
                                blk * NFREE : (blk + 1) * NFREE,
                            ],
                            in_=ot[:, blk * NFREE : (blk + 1) * NFREE],
                        )
    nc.compile()
    return nc


_LAST_RESULT = None  # BassKernelResults of the most recent run (for test.py)


def kernel(x, weight, bias, scale_x, lut=None, trace=False):
    global _LAST_RESULT
    sx = float(np.float32(scale_x))
    inv_sx = float(np.float32(1.0) / np.float32(scale_x))
    nc = build(inv_sx, sx)
    in_maps = [
        {
            "x": np.ascontiguousarray(x[i * BPC : (i + 1) * BPC], dtype=np.float32),
            "w": np.ascontiguousarray(weight, dtype=np.float32),
            "b": np.ascontiguousarray(bias, dtype=np.float32),
        }
        for i in range(NCORES)
    ]
    res = run_bass_kernel_spmd(nc, in_maps, core_ids=list(range(NCORES)), trace=trace)
    _LAST_RESULT = res
    return np.concatenate([r["out"] for r in res.results], axis=0)


# revision 10
# speedup vs baseline: 1.0098x; 1.0098x over previous
"""nn_Conv2d_BN_int8 TRN2 Bass kernel.

Reference computation (see harness reference.py):
    q_x = clip(round(x / scale_x), -127, 127)                    per-tensor int8
    scale_w = max|w| over (C,kH,kW) / 127;  q_w = clip(round(w / scale_w))
    acc = conv2d(q_x, q_w, 3x3, stride 1, pad 1)   (fp32 accumulation, integer-valued)
    out = acc * scale_x * scale_w[o] + bias[o]

Strategy:
  - Data-parallel over batch: 32 images -> 4 per core x 8 cores. Each core
    holds the full weights (1.2 MB).
  - int8 values in [-127,127] are exactly representable in bf16, and the PE
    accumulates in fp32 -> a bf16 GEMM of the quantized tensors is an exact
    integer GEMM (partial sums ~1e5 << 2^24).
  - conv 3x3/pad1 == sum over 9 taps of shifted GEMMs: K=C=128 (partition
    dim), M=O tile of 128, N=448 (8 output rows x 56 cols) accumulated in
    one PSUM bank over the 9 taps.
  - x is quantized on-device (DVE; round-to-nearest-even via the +1.5*2^23
    magic-number trick, matching jnp.round) into a zero-padded [128,58,58]
    bf16 image so every tap is a strided AP, no boundary cases.
  - w is quantized on-device in O-major layout (per-channel absmax reduce),
    then moved to C-major [c, tap, o] via 9 PE transposes per o-tile; the
    o-tile-1 prep is deferred until after the img0/o-tile-0 matmuls are
    emitted so the PE starts real work as early as possible.
  - dequant + bias fused into one DVE tensor_scalar pass (per-partition
    scale/bias APs) during PSUM eviction; output DMA per 8-row block.
"""

import numpy as np

import concourse.bacc as bacc
import concourse.mybir as mybir
import concourse.tile as tile
from concourse.bass_utils import run_bass_kernel_spmd
from concourse.masks import make_identity

B, C, H, W = 32, 128, 56, 56
O, KH, KW = 256, 3, 3
NCORES = 8
BPC = B // NCORES            # images per core
HP, WP = H + 2, W + 2        # padded image
RB = 8                       # output rows per matmul block
NBLK = H // RB               # 7 blocks
NFREE = RB * W               # 448 <= 512 (one PSUM bank)
NTAP = KH * KW               # 9
MAGIC = 12582912.0           # 1.5 * 2**23: fp32 RNE integer rounding trick
QMAX = 127.0

f32 = mybir.dt.float32
bf16 = mybir.dt.bfloat16


def build(inv_sx: float, sx: float):
    """Emit the per-core Bass program. inv_sx = 1/scale_x, sx = scale_x."""
    nc = bacc.Bacc(
        "TRN2",
        target_bir_lowering=False,
        debug=False,
        enable_asserts=False,
        num_devices=NCORES,
    )
    x_d = nc.dram_tensor("x", [BPC, C, H, W], f32, kind="ExternalInput").ap()
    w_d = nc.dram_tensor("w", [O, C, KH, KW], f32, kind="ExternalInput").ap()
    b_d = nc.dram_tensor("b", [O], f32, kind="ExternalInput").ap()
    out_d = nc.dram_tensor("out", [BPC, O, H, W], f32, kind="ExternalOutput").ap()

    # second HWDGE ring (qActDynamicHW) for input loads; outputs go on sync
    in_dma = nc.scalar if hasattr(nc.scalar, "dma_start") else nc.sync

    with tile.TileContext(nc) as tc:
        with (
            tc.tile_pool(name="singles", bufs=1) as singles,
            tc.tile_pool(name="wtmp", bufs=2) as wtmp,
            tc.tile_pool(name="wsc", bufs=8) as wsc,
            tc.tile_pool(name="qwp", bufs=2) as qwp,
            tc.tile_pool(name="xsp", bufs=2) as xsp,
            tc.tile_pool(name="xpadp", bufs=3) as xpadp,
            tc.tile_pool(name="outp", bufs=3) as outp,
            tc.tile_pool(name="psum", bufs=8, space="PSUM") as psump,
        ):
            idn = singles.tile([128, 128], bf16, name="idn")
            make_identity(nc, idn)

            # HAM warmup: ~40 dummy matmuls (idn @ idn) keep the PE busy
            # from ~t=0 so the clock gate is at 8/8 when real matmuls start
            wps = psump.tile([128, 128], f32, name="wps", tag="ps")
            for _ in range(40):
                nc.tensor.matmul(wps, idn, idn, start=True, stop=True)

            bias_sb = singles.tile([128, 2], f32, name="bias_sb")
            with tc.high_priority():
                nc.sync.dma_start(
                    out=bias_sb, in_=b_d.rearrange("(t p) -> p t", p=128)
                )

            # [c, tap, o] bf16 stationary weights for the GEMMs
            wmm = singles.tile([128, NTAP, O], bf16, name="wmm")
            # per-o dequant scale (scale_w * scale_x), partition layout
            dqs = singles.tile([128, 2], f32, name="dqs")

            # weight loads first on the sync ring (startup critical path);
            # x loads go on the scalar HWDGE ring so they never queue
            # behind weights or output stores
            w_flat = w_d.rearrange("o c kh kw -> o (c kh kw)")  # [256, 1152]
            wf = []
            with tc.high_priority():
                for t in range(2):
                    wf_t = wtmp.tile([128, C * NTAP], f32, name=f"wf{t}")
                    nc.sync.dma_start(
                        out=wf_t, in_=w_flat[t * 128 : (t + 1) * 128]
                    )
                    wf.append(wf_t)

            x_flat = x_d.rearrange("b c h w -> b c (h w)")      # [BPC, 128, 3136]
            out_flat = out_d.rearrange("b o h w -> b o (h w)")  # [BPC, 256, 3136]
            xs0 = xsp.tile([128, H * W], f32, name="xs")
            in_dma.dma_start(out=xs0, in_=x_flat[0])

            def prep_weights(t):
                """absmax -> 1/scale_w (Newton-refined) -> quantize -> PE
                transpose into wmm[:, :, t*128:(t+1)*128]."""
                mx = wsc.tile([128, 1], f32, name="mx")
                nc.vector.tensor_reduce(
                    out=mx, in_=wf[t], axis=mybir.AxisListType.X,
                    op=mybir.AluOpType.max, apply_absolute_value=True,
                )
                # rq = 127/maxabs, one Newton step: r1 = r0*(2 - mx*r0)
                r0 = wsc.tile([128, 1], f32, name="r0")
                nc.vector.reciprocal(out=r0, in_=mx)
                nt = wsc.tile([128, 1], f32, name="nt")
                nc.vector.tensor_mul(out=nt, in0=mx, in1=r0)
                nc.vector.tensor_scalar(
                    out=nt, in0=nt, scalar1=-1.0, scalar2=2.0,
                    op0=mybir.AluOpType.mult, op1=mybir.AluOpType.add,
                )
                rq = wsc.tile([128, 1], f32, name="rq")
                nc.vector.tensor_mul(out=rq, in0=r0, in1=nt)
                nc.vector.tensor_scalar(
                    out=rq, in0=rq, scalar1=QMAX, scalar2=None,
                    op0=mybir.AluOpType.mult,
                )
                # dequant scale = scale_w * scale_x = maxabs * (scale_x/127)
                nc.vector.tensor_scalar(
                    out=dqs[:, t : t + 1], in0=mx,
                    scalar1=float(np.float32(sx) / np.float32(QMAX)), scalar2=None,
                    op0=mybir.AluOpType.mult,
                )
                # q_w = clip(rne(w / s_w), -127, 127); rne via magic add/sub.
                # mult and magic-add must be separate instructions: a fused
                # tensor_scalar skips the intermediate fp32 rounding, which
                # changes which values sit on .5 rounding boundaries.
                nc.vector.tensor_scalar(
                    out=wf[t], in0=wf[t], scalar1=rq, scalar2=None,
                    op0=mybir.AluOpType.mult,
                )
                nc.vector.tensor_scalar(
                    out=wf[t], in0=wf[t], scalar1=MAGIC, scalar2=None,
                    op0=mybir.AluOpType.add,
                )
                nc.vector.tensor_scalar(
                    out=wf[t], in0=wf[t], scalar1=MAGIC, scalar2=QMAX,
                    op0=mybir.AluOpType.subtract, op1=mybir.AluOpType.min,
                )
                qw = qwp.tile([128, C * NTAP], bf16, name="qw")
                nc.vector.tensor_scalar(
                    out=qw, in0=wf[t], scalar1=-QMAX, scalar2=None,
                    op0=mybir.AluOpType.max,
                )
                # transpose [o, c] -> [c, o] per tap via the PE
                qw3 = qw.rearrange("o (c t) -> o c t", t=NTAP)
                for tap in range(NTAP):
                    tp = psump.tile([128, 128], bf16, name="tp", tag="ps")
                    nc.tensor.transpose(tp, qw3[:, :, tap], idn)
                    nc.scalar.copy(
                        out=wmm[:, tap, t * 128 : (t + 1) * 128], in_=tp
                    )

            prep_weights(0)

            for img in range(BPC):
                if img == 0:
                    xs = xs0
                else:
                    xs = xsp.tile([128, H * W], f32, name="xs")
                    in_dma.dma_start(out=xs, in_=x_flat[img])
                xp = xpadp.tile([128, HP, WP], bf16, name="xp")
                nc.gpsimd.memset(xp[:, 0, :], 0.0)
                nc.gpsimd.memset(xp[:, HP - 1, :], 0.0)
                nc.gpsimd.memset(xp[:, :, 0], 0.0)
                nc.gpsimd.memset(xp[:, :, WP - 1], 0.0)
                # q = clip(rne(x * (1/s_x)), -127, 127), in half-image
                # chunks so the first conv matmuls can start sooner.
                # mult / magic-add as separate instructions (see above).
                xs3 = xs.rearrange("c (h w) -> c h w", w=W)
                HH = H // 2
                for hh in range(2):
                    xsh = xs3[:, hh * HH : (hh + 1) * HH, :]
                    nc.vector.tensor_scalar(
                        out=xsh, in0=xsh, scalar1=inv_sx, scalar2=None,
                        op0=mybir.AluOpType.mult,
                    )
                    nc.vector.tensor_scalar(
                        out=xsh, in0=xsh, scalar1=MAGIC, scalar2=None,
                        op0=mybir.AluOpType.add,
                    )
                    nc.vector.tensor_scalar(
                        out=xsh, in0=xsh, scalar1=MAGIC, scalar2=QMAX,
                        op0=mybir.AluOpType.subtract, op1=mybir.AluOpType.min,
                    )
                    nc.vector.tensor_scalar(
                        out=xp[:, 1 + hh * HH : 1 + (hh + 1) * HH, 1 : W + 1],
                        in0=xsh,
                        scalar1=-QMAX, scalar2=None,
                        op0=mybir.AluOpType.max,
                    )
                for t in range(2):
                    if img == 0 and t == 1:
                        # deferred so img0/t0 matmuls precede it in the PE
                        # queue; ready well before its own matmuls need it
                        prep_weights(1)
                    ot = outp.tile([128, H * W], f32, name="ot")
                    for blk in range(NBLK):
                        ps = psump.tile([128, NFREE], f32, name="ps", tag="ps")
                        for tap in range(NTAP):
                            dy, dx = divmod(tap, KW)
                            nc.tensor.matmul(
                                ps,
                                wmm[:, tap, t * 128 : (t + 1) * 128],
                                xp[:, blk * RB + dy : blk * RB + dy + RB, dx : dx + W],
                                start=(tap == 0),
                                stop=(tap == NTAP - 1),
                            )
                        # out = acc * (s_x*s_w[o]) + bias[o]  (DVE evict)
                        nc.vector.tensor_scalar(
                            out=ot[:, blk * NFREE : (blk + 1) * NFREE],
                            in0=ps,
                            scalar1=dqs[:, t : t + 1],
                            scalar2=bias_sb[:, t : t + 1],
                            op0=mybir.AluOpType.mult,
                            op1=mybir.AluOpType.add,
                        )
                        nc.sync.dma_start(
                            out=out_flat[
                                img, t * 128 : (t + 1) * 128,
                                blk * NFREE : (blk + 1) * NFREE,
                            ],
                            in_=ot[:, blk * NFREE : (blk + 1) * NFREE],
                        )
    nc.compile()
    return nc


_LAST_RESULT = None  # BassKernelResults of the most recent run (for test.py)


def kernel(x, weight, bias, scale_x, lut=None, trace=False):
    global _LAST_RESULT
    sx = float(np.float32(scale_x))
    inv_sx = float(np.float32(1.0) / np.float32(scale_x))
    nc = build(inv_sx, sx)
    in_maps = [
        {
            "x": np.ascontiguousarray(x[i * BPC : (i + 1) * BPC], dtype=np.float32),
            "w": np.ascontiguousarray(weight, dtype=np.float32),
            "b": np.ascontiguousarray(bias, dtype=np.float32),
        }
        for i in range(NCORES)
    ]
    res = run_bass_kernel_spmd(nc, in_maps, core_ids=list(range(NCORES)), trace=trace)
    _LAST_RESULT = res
    return np.concatenate([r["out"] for r in res.results], axis=0)


# revision 12
# speedup vs baseline: 1.0666x; 1.0562x over previous
"""nn_Conv2d_BN_int8 TRN2 Bass kernel.

Reference computation (see harness reference.py):
    q_x = clip(round(x / scale_x), -127, 127)                    per-tensor int8
    scale_w = max|w| over (C,kH,kW) / 127;  q_w = clip(round(w / scale_w))
    acc = conv2d(q_x, q_w, 3x3, stride 1, pad 1)   (fp32 accumulation, integer-valued)
    out = acc * scale_x * scale_w[o] + bias[o]

Strategy:
  - Data-parallel over batch: 32 images -> 4 per core x 8 cores. Each core
    holds the full weights (1.2 MB).
  - int8 values in [-127,127] are exactly representable in bf16, and the PE
    accumulates in fp32 -> a bf16 GEMM of the quantized tensors is an exact
    integer GEMM (partial sums ~1e5 << 2^24).
  - conv 3x3/pad1 == sum over 9 taps of shifted GEMMs: K=C=128 (partition
    dim), M=O tile of 128, N=448 (8 output rows x 56 cols) accumulated in
    one PSUM bank over the 9 taps.
  - x is quantized on-device (DVE; round-to-nearest-even via the +1.5*2^23
    magic-number trick, matching jnp.round) into a zero-padded [128,58,58]
    bf16 image so every tap is a strided AP, no boundary cases.
  - w is quantized on-device in O-major layout (per-channel absmax reduce),
    then moved to C-major [c, tap, o] via 9 PE transposes per o-tile; the
    o-tile-1 prep is deferred until after the img0/o-tile-0 matmuls are
    emitted so the PE starts real work as early as possible.
  - dequant + bias fused into one DVE tensor_scalar pass (per-partition
    scale/bias APs) during PSUM eviction; output DMA per 8-row block.
"""

import numpy as np

import concourse.bacc as bacc
import concourse.mybir as mybir
import concourse.tile as tile
from concourse.bass_utils import run_bass_kernel_spmd
from concourse.masks import make_identity

B, C, H, W = 32, 128, 56, 56
O, KH, KW = 256, 3, 3
NCORES = 8
BPC = B // NCORES            # images per core
HP, WP = H + 2, W + 2        # padded image
RB = 8                       # output rows per matmul block
NBLK = H // RB               # 7 blocks
NFREE = RB * W               # 448 <= 512 (one PSUM bank)
NTAP = KH * KW               # 9
MAGIC = 12582912.0           # 1.5 * 2**23: fp32 RNE integer rounding trick
QMAX = 127.0

f32 = mybir.dt.float32
bf16 = mybir.dt.bfloat16


def build(inv_sx: float, sx: float):
    """Emit the per-core Bass program. inv_sx = 1/scale_x, sx = scale_x."""
    nc = bacc.Bacc(
        "TRN2",
        target_bir_lowering=False,
        debug=False,
        enable_asserts=False,
        num_devices=NCORES,
    )
    x_d = nc.dram_tensor("x", [BPC, C, H, W], f32, kind="ExternalInput").ap()
    w_d = nc.dram_tensor("w", [O, C, KH, KW], f32, kind="ExternalInput").ap()
    b_d = nc.dram_tensor("b", [O], f32, kind="ExternalInput").ap()
    out_d = nc.dram_tensor("out", [BPC, O, H, W], f32, kind="ExternalOutput").ap()

    # second HWDGE ring (qActDynamicHW) for input loads; outputs go on sync
    in_dma = nc.scalar if hasattr(nc.scalar, "dma_start") else nc.sync

    with tile.TileContext(nc) as tc:
        with (
            tc.tile_pool(name="singles", bufs=1) as singles,
            tc.tile_pool(name="wtmp", bufs=2) as wtmp,
            tc.tile_pool(name="wsc", bufs=8) as wsc,
            tc.tile_pool(name="qwp", bufs=2) as qwp,
            tc.tile_pool(name="xsp", bufs=2) as xsp,
            tc.tile_pool(name="xpadp", bufs=3) as xpadp,
            tc.tile_pool(name="outp", bufs=3) as outp,
            tc.tile_pool(name="psum", bufs=8, space="PSUM") as psump,
        ):
            idn = singles.tile([128, 128], bf16, name="idn")
            make_identity(nc, idn)

            # HAM warmup: ~40 dummy matmuls (idn @ idn) keep the PE busy
            # from ~t=0 so the clock gate is at 8/8 when real matmuls start
            wps = psump.tile([128, 128], f32, name="wps", tag="ps")
            for _ in range(40):
                nc.tensor.matmul(wps, idn, idn, start=True, stop=True)

            bias_sb = singles.tile([128, 2], f32, name="bias_sb")
            with tc.high_priority():
                nc.sync.dma_start(
                    out=bias_sb, in_=b_d.rearrange("(t p) -> p t", p=128)
                )

            # [c, tap, o] bf16 stationary weights for the GEMMs
            wmm = singles.tile([128, NTAP, O], bf16, name="wmm")
            # per-o dequant scale (scale_w * scale_x), partition layout
            dqs = singles.tile([128, 2], f32, name="dqs")

            # weight loads first on the sync ring (startup critical path);
            # x loads go on the scalar HWDGE ring so they never queue
            # behind weights or output stores
            w_flat = w_d.rearrange("o c kh kw -> o (c kh kw)")  # [256, 1152]
            wf = []
            wdma = []
            with tc.high_priority():
                for t in range(2):
                    wf_t = wtmp.tile([128, C * NTAP], f32, name=f"wf{t}")
                    wdma.append(
                        nc.sync.dma_start(
                            out=wf_t, in_=w_flat[t * 128 : (t + 1) * 128]
                        )
                    )
                    wf.append(wf_t)

            x_flat = x_d.rearrange("b c h w -> b c (h w)")      # [BPC, 128, 3136]
            out_flat = out_d.rearrange("b o h w -> b o (h w)")  # [BPC, 256, 3136]
            xs0 = xsp.tile([128, H * W], f32, name="xs")
            x0di = in_dma.dma_start(out=xs0, in_=x_flat[0])
            # hard-order the first x load after the w loads: both rings share
            # the 16 SDMA engines at packet granularity, and 1.6MB x packets
            # starve the startup-critical 0.6MB weight transfers otherwise
            from concourse.tile_rust import add_dep_helper

            add_dep_helper(
                x0di.ins, wdma[0].ins, sync=True,
                reason="x0 load yields SDMA service to weight loads",
            )

            def prep_weights(t):
                """absmax -> 1/scale_w (Newton-refined) -> quantize -> PE
                transpose into wmm[:, :, t*128:(t+1)*128]."""
                mx = wsc.tile([128, 1], f32, name="mx")
                nc.vector.tensor_reduce(
                    out=mx, in_=wf[t], axis=mybir.AxisListType.X,
                    op=mybir.AluOpType.max, apply_absolute_value=True,
                )
                # rq = 127/maxabs, one Newton step: r1 = r0*(2 - mx*r0)
                r0 = wsc.tile([128, 1], f32, name="r0")
                nc.vector.reciprocal(out=r0, in_=mx)
                nt = wsc.tile([128, 1], f32, name="nt")
                nc.vector.tensor_mul(out=nt, in0=mx, in1=r0)
                nc.vector.tensor_scalar(
                    out=nt, in0=nt, scalar1=-1.0, scalar2=2.0,
                    op0=mybir.AluOpType.mult, op1=mybir.AluOpType.add,
                )
                rq = wsc.tile([128, 1], f32, name="rq")
                nc.vector.tensor_mul(out=rq, in0=r0, in1=nt)
                nc.vector.tensor_scalar(
                    out=rq, in0=rq, scalar1=QMAX, scalar2=None,
                    op0=mybir.AluOpType.mult,
                )
                # dequant scale = scale_w * scale_x = maxabs * (scale_x/127)
                nc.vector.tensor_scalar(
                    out=dqs[:, t : t + 1], in0=mx,
                    scalar1=float(np.float32(sx) / np.float32(QMAX)), scalar2=None,
                    op0=mybir.AluOpType.mult,
                )
                # q_w = clip(rne(w / s_w), -127, 127); rne via magic add/sub.
                # mult and magic-add must be separate instructions: a fused
                # tensor_scalar skips the intermediate fp32 rounding, which
                # changes which values sit on .5 rounding boundaries.
                nc.vector.tensor_scalar(
                    out=wf[t], in0=wf[t], scalar1=rq, scalar2=None,
                    op0=mybir.AluOpType.mult,
                )
                nc.vector.tensor_scalar(
                    out=wf[t], in0=wf[t], scalar1=MAGIC, scalar2=None,
                    op0=mybir.AluOpType.add,
                )
                nc.vector.tensor_scalar(
                    out=wf[t], in0=wf[t], scalar1=MAGIC, scalar2=QMAX,
                    op0=mybir.AluOpType.subtract, op1=mybir.AluOpType.min,
                )
                qw = qwp.tile([128, C * NTAP], bf16, name="qw")
                nc.vector.tensor_scalar(
                    out=qw, in0=wf[t], scalar1=-QMAX, scalar2=None,
                    op0=mybir.AluOpType.max,
                )
                # transpose [o, c] -> [c, o] per tap via the PE
                qw3 = qw.rearrange("o (c t) -> o c t", t=NTAP)
                for tap in range(NTAP):
                    tp = psump.tile([128, 128], bf16, name="tp", tag="ps")
                    nc.tensor.transpose(tp, qw3[:, :, tap], idn)
                    nc.scalar.copy(
                        out=wmm[:, tap, t * 128 : (t + 1) * 128], in_=tp
                    )

            prep_weights(0)

            for img in range(BPC):
                if img == 0:
                    xs = xs0
                else:
                    xs = xsp.tile([128, H * W], f32, name="xs")
                    in_dma.dma_start(out=xs, in_=x_flat[img])
                xp = xpadp.tile([128, HP, WP], bf16, name="xp")
                nc.gpsimd.memset(xp[:, 0, :], 0.0)
                nc.gpsimd.memset(xp[:, HP - 1, :], 0.0)
                nc.gpsimd.memset(xp[:, :, 0], 0.0)
                nc.gpsimd.memset(xp[:, :, WP - 1], 0.0)
                # q = clip(rne(x * (1/s_x)), -127, 127), in half-image
                # chunks so the first conv matmuls can start sooner.
                # mult / magic-add as separate instructions (see above).
                xs3 = xs.rearrange("c (h w) -> c h w", w=W)
                HH = H // 2
                for hh in range(2):
                    xsh = xs3[:, hh * HH : (hh + 1) * HH, :]
                    nc.vector.tensor_scalar(
                        out=xsh, in0=xsh, scalar1=inv_sx, scalar2=None,
                        op0=mybir.AluOpType.mult,
                    )
                    nc.vector.tensor_scalar(
                        out=xsh, in0=xsh, scalar1=MAGIC, scalar2=None,
                        op0=mybir.AluOpType.add,
                    )
                    nc.vector.tensor_scalar(
                        out=xsh, in0=xsh, scalar1=MAGIC, scalar2=QMAX,
                        op0=mybir.AluOpType.subtract, op1=mybir.AluOpType.min,
                    )
                    nc.vector.tensor_scalar(
                        out=xp[:, 1 + hh * HH : 1 + (hh + 1) * HH, 1 : W + 1],
                        in0=xsh,
                        scalar1=-QMAX, scalar2=None,
                        op0=mybir.AluOpType.max,
                    )
                for t in range(2):
                    if img == 0 and t == 1:
                        # deferred so img0/t0 matmuls precede it in the PE
                        # queue; ready well before its own matmuls need it
                        prep_weights(1)
                    ot = outp.tile([128, H * W], f32, name="ot")
                    for blk in range(NBLK):
                        ps = psump.tile([128, NFREE], f32, name="ps", tag="ps")
                        for tap in range(NTAP):
                            dy, dx = divmod(tap, KW)
                            nc.tensor.matmul(
                                ps,
                                wmm[:, tap, t * 128 : (t + 1) * 128],
                                xp[:, blk * RB + dy : blk * RB + dy + RB, dx : dx + W],
                                start=(tap == 0),
                                stop=(tap == NTAP - 1),
                            )
                        # out = acc * (s_x*s_w[o]) + bias[o]  (DVE evict)
                        nc.vector.tensor_scalar(
                            out=ot[:, blk * NFREE : (blk + 1) * NFREE],
                            in0=ps,
                            scalar1=dqs[:, t : t + 1],
                            scalar2=bias_sb[:, t : t + 1],
                            op0=mybir.AluOpType.mult,
                            op1=mybir.AluOpType.add,
                        )
                        nc.sync.dma_start(
                            out=out_flat[
                                img, t * 128 : (t + 1) * 128,
                                blk * NFREE : (blk + 1) * NFREE,
                            ],
                            in_=ot[:, blk * NFREE : (blk + 1) * NFREE],
                        )
    nc.compile()
    return nc


_LAST_RESULT = None  # BassKernelResults of the most recent run (for test.py)


def kernel(x, weight, bias, scale_x, lut=None, trace=False):
    global _LAST_RESULT
    sx = float(np.float32(scale_x))
    inv_sx = float(np.float32(1.0) / np.float32(scale_x))
    nc = build(inv_sx, sx)
    in_maps = [
        {
            "x": np.ascontiguousarray(x[i * BPC : (i + 1) * BPC], dtype=np.float32),
            "w": np.ascontiguousarray(weight, dtype=np.float32),
            "b": np.ascontiguousarray(bias, dtype=np.float32),
        }
        for i in range(NCORES)
    ]
    res = run_bass_kernel_spmd(nc, in_maps, core_ids=list(range(NCORES)), trace=trace)
    _LAST_RESULT = res
    return np.concatenate([r["out"] for r in res.results], axis=0)


# revision 16
# speedup vs baseline: 1.0819x; 1.0143x over previous
"""nn_Conv2d_BN_int8 TRN2 Bass kernel.

Reference computation (see harness reference.py):
    q_x = clip(round(x / scale_x), -127, 127)                    per-tensor int8
    scale_w = max|w| over (C,kH,kW) / 127;  q_w = clip(round(w / scale_w))
    acc = conv2d(q_x, q_w, 3x3, stride 1, pad 1)   (fp32 accumulation, integer-valued)
    out = acc * scale_x * scale_w[o] + bias[o]

Strategy:
  - Data-parallel over batch: 32 images -> 4 per core x 8 cores. Each core
    holds the full weights (1.2 MB).
  - int8 values in [-127,127] are exactly representable in bf16, and the PE
    accumulates in fp32 -> a bf16 GEMM of the quantized tensors is an exact
    integer GEMM (partial sums ~1e5 << 2^24).
  - conv 3x3/pad1 == sum over 9 taps of shifted GEMMs: K=C=128 (partition
    dim), M=O tile of 128, N=448 (8 output rows x 56 cols) accumulated in
    one PSUM bank over the 9 taps.
  - x is quantized on-device (DVE; round-to-nearest-even via the +1.5*2^23
    magic-number trick, matching jnp.round) into a zero-padded [128,58,58]
    bf16 image so every tap is a strided AP, no boundary cases.
  - w is quantized on-device in O-major layout (per-channel absmax reduce),
    then moved to C-major [c, tap, o] via 9 PE transposes per o-tile; the
    o-tile-1 prep is deferred until after the img0/o-tile-0 matmuls are
    emitted so the PE starts real work as early as possible.
  - dequant + bias fused into one DVE tensor_scalar pass (per-partition
    scale/bias APs) during PSUM eviction; output DMA per 8-row block.
"""

import numpy as np

import concourse.bacc as bacc
import concourse.mybir as mybir
import concourse.tile as tile
from concourse.bass_utils import run_bass_kernel_spmd
from concourse.masks import make_identity

B, C, H, W = 32, 128, 56, 56
O, KH, KW = 256, 3, 3
NCORES = 8
BPC = B // NCORES            # images per core
HP, WP = H + 2, W + 2        # padded image
RB = 8                       # output rows per matmul block
NBLK = H // RB               # 7 blocks
NFREE = RB * W               # 448 <= 512 (one PSUM bank)
NTAP = KH * KW               # 9
MAGIC = 12582912.0           # 1.5 * 2**23: fp32 RNE integer rounding trick
QMAX = 127.0

f32 = mybir.dt.float32
bf16 = mybir.dt.bfloat16


def build(inv_sx: float, sx: float):
    """Emit the per-core Bass program. inv_sx = 1/scale_x, sx = scale_x."""
    nc = bacc.Bacc(
        "TRN2",
        target_bir_lowering=False,
        debug=False,
        enable_asserts=False,
        num_devices=NCORES,
    )
    x_d = nc.dram_tensor("x", [BPC, C, H, W], f32, kind="ExternalInput").ap()
    w_d = nc.dram_tensor("w", [O, C, KH, KW], f32, kind="ExternalInput").ap()
    b_d = nc.dram_tensor("b", [O], f32, kind="ExternalInput").ap()
    out_d = nc.dram_tensor("out", [BPC, O, H, W], f32, kind="ExternalOutput").ap()

    # second HWDGE ring (qActDynamicHW) for input loads; outputs go on sync
    in_dma = nc.scalar if hasattr(nc.scalar, "dma_start") else nc.sync

    with tile.TileContext(nc) as tc:
        with (
            tc.tile_pool(name="singles", bufs=1) as singles,
            tc.tile_pool(name="wtmp", bufs=2) as wtmp,
            tc.tile_pool(name="wsc", bufs=8) as wsc,
            tc.tile_pool(name="qwp", bufs=2) as qwp,
            tc.tile_pool(name="xsp", bufs=2) as xsp,
            tc.tile_pool(name="xpadp", bufs=3) as xpadp,
            tc.tile_pool(name="outp", bufs=3) as outp,
            tc.tile_pool(name="psum", bufs=8, space="PSUM") as psump,
        ):
            idn = singles.tile([128, 128], bf16, name="idn")
            make_identity(nc, idn)

            bias_sb = singles.tile([128, 2], f32, name="bias_sb")

            # [c, tap, o] bf16 stationary weights for the GEMMs
            wmm = singles.tile([128, NTAP, O], bf16, name="wmm")
            # per-o dequant scale (scale_w * scale_x), partition layout
            dqs = singles.tile([128, 2], f32, name="dqs")

            # weight loads first on the sync ring (startup critical path);
            # x loads go on the scalar HWDGE ring so they never queue
            # behind weights or output stores
            w_flat = w_d.rearrange("o c kh kw -> o (c kh kw)")  # [256, 1152]
            wf = []
            wdma = []
            with tc.high_priority():
                for t in range(2):
                    wf_t = wtmp.tile([128, C * NTAP], f32, name=f"wf{t}")
                    wdma.append(
                        nc.sync.dma_start(
                            out=wf_t, in_=w_flat[t * 128 : (t + 1) * 128]
                        )
                    )
                    wf.append(wf_t)

            x_flat = x_d.rearrange("b c h w -> b c (h w)")      # [BPC, 128, 3136]
            out_flat = out_d.rearrange("b o h w -> b o (h w)")  # [BPC, 256, 3136]
            # img0's x load on the SAME sync ring, FIFO behind the w loads,
            # so the 1.6MB x transfer can't starve the startup-critical
            # weight transfers at the shared SDMA engines. bias after.
            xs0 = xsp.tile([128, H * W], f32, name="xs")
            x0di = nc.sync.dma_start(out=xs0, in_=x_flat[0])
            nc.sync.dma_start(out=bias_sb, in_=b_d.rearrange("(t p) -> p t", p=128))

            def prep_weights(t):
                """absmax -> 1/scale_w (Newton-refined) -> quantize -> PE
                transpose into wmm[:, :, t*128:(t+1)*128]."""
                mx = wsc.tile([128, 1], f32, name="mx")
                nc.vector.tensor_reduce(
                    out=mx, in_=wf[t], axis=mybir.AxisListType.X,
                    op=mybir.AluOpType.max, apply_absolute_value=True,
                )
                # rq = 127/maxabs, one Newton step: r1 = r0*(2 - mx*r0)
                r0 = wsc.tile([128, 1], f32, name="r0")
                nc.vector.reciprocal(out=r0, in_=mx)
                nt = wsc.tile([128, 1], f32, name="nt")
                nc.vector.tensor_mul(out=nt, in0=mx, in1=r0)
                nc.vector.tensor_scalar(
                    out=nt, in0=nt, scalar1=-1.0, scalar2=2.0,
                    op0=mybir.AluOpType.mult, op1=mybir.AluOpType.add,
                )
                rq = wsc.tile([128, 1], f32, name="rq")
                nc.vector.tensor_mul(out=rq, in0=r0, in1=nt)
                nc.vector.tensor_scalar(
                    out=rq, in0=rq, scalar1=QMAX, scalar2=None,
                    op0=mybir.AluOpType.mult,
                )
                # dequant scale = scale_w * scale_x = maxabs * (scale_x/127)
                nc.vector.tensor_scalar(
                    out=dqs[:, t : t + 1], in0=mx,
                    scalar1=float(np.float32(sx) / np.float32(QMAX)), scalar2=None,
                    op0=mybir.AluOpType.mult,
                )
                # q_w = clip(rne(w / s_w), -127, 127); rne via magic add/sub.
                # mult and magic-add must be separate instructions: a fused
                # tensor_scalar skips the intermediate fp32 rounding, which
                # changes which values sit on .5 rounding boundaries.
                nc.vector.tensor_scalar(
                    out=wf[t], in0=wf[t], scalar1=rq, scalar2=None,
                    op0=mybir.AluOpType.mult,
                )
                nc.vector.tensor_scalar(
                    out=wf[t], in0=wf[t], scalar1=MAGIC, scalar2=None,
                    op0=mybir.AluOpType.add,
                )
                nc.vector.tensor_scalar(
                    out=wf[t], in0=wf[t], scalar1=MAGIC, scalar2=QMAX,
                    op0=mybir.AluOpType.subtract, op1=mybir.AluOpType.min,
                )
                qw = qwp.tile([128, C * NTAP], bf16, name="qw")
                nc.vector.tensor_scalar(
                    out=qw, in0=wf[t], scalar1=-QMAX, scalar2=None,
                    op0=mybir.AluOpType.max,
                )
                # transpose [o, c] -> [c, o] per tap via the PE
                qw3 = qw.rearrange("o (c t) -> o c t", t=NTAP)
                for tap in range(NTAP):
                    tp = psump.tile([128, 128], bf16, name="tp", tag="ps")
                    nc.tensor.transpose(tp, qw3[:, :, tap], idn)
                    nc.scalar.copy(
                        out=wmm[:, tap, t * 128 : (t + 1) * 128], in_=tp
                    )

            prep_weights(0)

            # x-wait filler: a few dummy matmuls between the transposes and
            # the first conv keep the PE busy (HAM clock gate at 8/8) while
            # img0's quantization finishes on the DVE
            wps = psump.tile([128, 128], f32, name="wps", tag="ps")
            for _ in range(16):
                nc.tensor.matmul(wps, idn, idn, start=True, stop=True)

            from concourse.tile_rust import add_dep_helper

            for img in range(BPC):
                if img == 0:
                    xs = xs0
                else:
                    xs = xsp.tile([128, H * W], f32, name="xs")
                    xdi = in_dma.dma_start(out=xs, in_=x_flat[img])
                    if img == 1:
                        # keep the scalar-ring x loads out of the startup
                        # window entirely
                        add_dep_helper(
                            xdi.ins, x0di.ins, sync=True,
                            reason="x1 load starts after x0 load",
                        )
                xp = xpadp.tile([128, HP, WP], bf16, name="xp")
                nc.gpsimd.memset(xp[:, 0, :], 0.0)
                nc.gpsimd.memset(xp[:, HP - 1, :], 0.0)
                nc.gpsimd.memset(xp[:, :, 0], 0.0)
                nc.gpsimd.memset(xp[:, :, WP - 1], 0.0)
                # q = clip(rne(x * (1/s_x)), -127, 127), in half-image
                # chunks so the first conv matmuls can start sooner.
                # mult / magic-add as separate instructions (see above).
                xs3 = xs.rearrange("c (h w) -> c h w", w=W)
                HH = H // 2
                for hh in range(2):
                    xsh = xs3[:, hh * HH : (hh + 1) * HH, :]
                    nc.vector.tensor_scalar(
                        out=xsh, in0=xsh, scalar1=inv_sx, scalar2=None,
                        op0=mybir.AluOpType.mult,
                    )
                    nc.vector.tensor_scalar(
                        out=xsh, in0=xsh, scalar1=MAGIC, scalar2=None,
                        op0=mybir.AluOpType.add,
                    )
                    nc.vector.tensor_scalar(
                        out=xsh, in0=xsh, scalar1=MAGIC, scalar2=QMAX,
                        op0=mybir.AluOpType.subtract, op1=mybir.AluOpType.min,
                    )
                    nc.vector.tensor_scalar(
                        out=xp[:, 1 + hh * HH : 1 + (hh + 1) * HH, 1 : W + 1],
                        in0=xsh,
                        scalar1=-QMAX, scalar2=None,
                        op0=mybir.AluOpType.max,
                    )
                for t in range(2):
                    if img == 0 and t == 1:
                        # deferred so img0/t0 matmuls precede it in the PE
                        # queue; ready well before its own matmuls need it
                        prep_weights(1)
                    ot = outp.tile([128, H * W], f32, name="ot")
                    for blk in range(NBLK):
                        ps = psump.tile([128, NFREE], f32, name="ps", tag="ps")
                        for tap in range(NTAP):
                            dy, dx = divmod(tap, KW)
                            nc.tensor.matmul(
                                ps,
                                wmm[:, tap, t * 128 : (t + 1) * 128],
                                xp[:, blk * RB + dy : blk * RB + dy + RB, dx : dx + W],
                                start=(tap == 0),
                                stop=(tap == NTAP - 1),
                            )
                        # out = acc * (s_x*s_w[o]) + bias[o]  (DVE evict)
                        nc.vector.tensor_scalar(
                            out=ot[:, blk * NFREE : (blk + 1) * NFREE],
                            in0=ps,
                            scalar1=dqs[:, t : t + 1],
                            scalar2=bias_sb[:, t : t + 1],
                            op0=mybir.AluOpType.mult,
                            op1=mybir.AluOpType.add,
                        )
                        nc.sync.dma_start(
                            out=out_flat[
                                img, t * 128 : (t + 1) * 128,
                                blk * NFREE : (blk + 1) * NFREE,
                            ],
                            in_=ot[:, blk * NFREE : (blk + 1) * NFREE],
                        )
    nc.compile()
    return nc


_LAST_RESULT = None  # BassKernelResults of the most recent run (for test.py)


def kernel(x, weight, bias, scale_x, lut=None, trace=False):
    global _LAST_RESULT
    sx = float(np.float32(scale_x))
    inv_sx = float(np.float32(1.0) / np.float32(scale_x))
    nc = build(inv_sx, sx)
    in_maps = [
        {
            "x": np.ascontiguousarray(x[i * BPC : (i + 1) * BPC], dtype=np.float32),
            "w": np.ascontiguousarray(weight, dtype=np.float32),
            "b": np.ascontiguousarray(bias, dtype=np.float32),
        }
        for i in range(NCORES)
    ]
    res = run_bass_kernel_spmd(nc, in_maps, core_ids=list(range(NCORES)), trace=trace)
    _LAST_RESULT = res
    return np.concatenate([r["out"] for r in res.results], axis=0)
